# revision 41
# baseline (speedup 1.0000x reference)
"""BSplineKAN layer kernel for 8 Trainium2 NeuronCores.

Math
----
Per element x (xc = clip(x, -1, 1)) the reference computes
    y[n,o] = sum_{i,b} basis_b(xc[n,i]) * coeff[o,i,b] + silu(xc) @ w_base.T + bias
with the 7-function clamped cubic B-spline basis on knots
{-1(x4), -0.5, 0, 0.5, 1(x4)}.  Reference quirk: at xc == 1.0 exactly the
basis row is all ZERO.

On [-1, 1) the basis lives in the 7-dim space spanned by the truncated-power
features of xc
    phi = [1, x, x^2, x^3, relu(-x-0.5)^3, relu(x)^3, relu(x-0.5)^3]
(exact integer/48 conversion matrix T, hardcoded below).  Linear tricks
remove all masking from the device:
  * the constant feature's contribution sum_i W0[i,o] is added on the HOST
    (with the input bias), so phi0 never hits the matmul;
  * the xc == 1 edge case becomes one extra linear feature e = (x >= 1)
    whose weights cancel the spline's 1^- limit:
    W_e = -(W0 + sum_f phi_f(1) * fp16(W_f)), folded against the fp16-rounded
    weights so the cancellation is exact at working resolution.  The edge
    decision is made on the HOST in f32 (f16 rounding must not flip the
    branch) and shipped as a 2.0 sentinel inside the f16 x tensor.

Contraction layout (K = 8 * 1024):
  * 6 truncated-power features run as 48 fp16 K-tiles (their weights carry
    ~5x term cancellation, so fp8 anywhere on them measures 3-25e-2 error
    vs the 2e-2 gate -- evaluated and rejected);
  * e and silu run as 16 fp8e4 DoubleRow pairs (0.5 cycles/row) carrying
    Dekker-split weights: pair = (F, F/16) x (q8(W), q8(16*(W - q8(W)))),
    giving ~fp16 weight precision at half the PE cost.  e is 0/1 (fp8
    exact); silu's fp8 feature rounding adds ~2e-3 relative error.
  PE time: 48*512 + 16*256 cycles per 128-batch group = 191us/core vs
  300us for the 11-feature fp16 baseline.

Distribution: 4-way batch x 2-way d_out mesh over 8 cores.  Per core:
x host-encoded (1024, 2048) f16, weights (6144, 512) f16 + (4096, 512) f8
resident in SBUF, output (2048, 512) f32; y += host bias after gather.
"""

import numpy as np

# ---- problem constants (hardcoded per contract) ----
N_FULL, D_IN, D_OUT = 8192, 1024, 1024
MESH_N, MESH_O = 4, 2                 # 4-way batch x 2-way d_out
N_SHARD = N_FULL // MESH_N            # 2048
O_SHARD = D_OUT // MESH_O             # 512
P = 128
NF16 = 4                              # fp16 feature planes (phi_1,4,5,6)
IB = D_IN // P                        # 8 i-blocks
KT16 = NF16 * IB                      # 32 fp16 K-tiles
# fp8 DoubleRow pairs: A-pairs x2, x3 (8+8), B-pairs x2, x3 (4+4),
# e (8, Dekker weights), silu (4, cross-ib pairs w/ single fp8 weights --
# w_base sigma 1/32 tolerates plain fp8) = 36
NPAIR = 36
NCHUNK = 256                          # batch cols per pipeline chunk
WARMUP = 120                          # PE clock-ramp filler matmuls
NSUB = NCHUNK // P                    # 2
CHUNKS = N_SHARD // NCHUNK            # 8

# basis_b(x) = sum_f T48[f, b]/48 * phi_f(x) on [-1, 1),
# phi = [1, x, x^2, x^3, relu(-x-0.5)^3, relu(x)^3, relu(x-0.5)^3]
_T48 = np.array([
    [0,    0,    8,    32,   8,    0,    0],
    [0,    0,   -48,   0,    48,   0,    0],
    [0,    0,    96,  -192,  96,   0,    0],
    [0,   -96,   224, -192,  64,   0,    0],
    [384, -768,  576, -256,  64,   0,    0],
    [0,    96,  -288,  384, -288,  96,   0],
    [0,    0,    64,  -256,  576, -768,  384],
], dtype=np.float64)

# fp16 plane order (K-tile index f*IB + ib); phi index it maps to:
#   f0: xc (phi1)   f1: xc^2 (phi2)        f2: xc^3 (phi3)
#   f3: relu(-(x+.5)^3) (phi4)  f4: relu(x^3) (phi5)  f5: relu((x-.5)^3) (phi6)
_PHI_AT_1 = np.array([1.0, 1.0, 1.0, 0.0, 1.0, 0.125])  # phi_{1..6}(1)

_PROGRAM = None  # compiled Bass program, built once


def _build_program():
    import concourse.mybir as mybir
    import concourse.tile as tile
    from concourse import bacc

    f32 = mybir.dt.float32
    f16 = mybir.dt.float16
    f8 = mybir.dt.float8e4
    Op = mybir.AluOpType
    Act = mybir.ActivationFunctionType
    DR = mybir.MatmulPerfMode.DoubleRow

    nc = bacc.Bacc("TRN2", target_bir_lowering=False, debug=False)
    xt_d = nc.dram_tensor("xt", [D_IN, N_SHARD], f16, kind="ExternalInput").ap()
    w_d = nc.dram_tensor("wt", [KT16 * P, O_SHARD], f16, kind="ExternalInput").ap()
    w8_d = nc.dram_tensor("wt8", [2 * NPAIR * P, O_SHARD], f8,
                          kind="ExternalInput").ap()
    y_d = nc.dram_tensor("y", [N_SHARD, O_SHARD], f32, kind="ExternalOutput").ap()

    with tile.TileContext(nc) as tc:
        with (
            tc.tile_pool(name="const", bufs=1) as const_pool,
            tc.tile_pool(name="wt", bufs=1) as wt_pool,
            tc.tile_pool(name="feat", bufs=2) as f_pool,
            tc.tile_pool(name="xc", bufs=3) as xc_pool,
            tc.tile_pool(name="tmp", bufs=2) as tmp_pool,
            tc.tile_pool(name="out", bufs=2) as out_pool,
            tc.tile_pool(name="pso", bufs=4, space="PSUM") as psum_out,
        ):
            # warm-up tile memset FIRST so PE warm-up matmuls start asap and
            # anchor the p-state clock ramp; they fill until the first weight
            # slab + x chunk land (~5.3us)
            wz = const_pool.tile([P, P], f16, name="wz")
            nc.gpsimd.memset(wz[:], 0.0)
            pw = psum_out.tile([P, 64], f32, tag="pwarm", name="pwarm", bufs=1)
            for i in range(WARMUP):
                nc.tensor.matmul(pw[:], wz[:], wz[:, :64],
                                 start=(i == 0), stop=(i == WARMUP - 1))

            # tiny dummy activations so both ACT table sets load during the
            # initial DMA wait instead of on the first feature's critical path
            warm = const_pool.tile([P, 1], f32, name="warm")
            nc.gpsimd.memset(warm[:], 0.0)
            nc.scalar.activation(warm[:], warm[:], Act.Square)
            nc.scalar.activation(warm[:], warm[:], Act.Silu)
            b05 = const_pool.tile([P, 1], f32, name="b05")
            nc.gpsimd.memset(b05[:], 0.5)
            bm05 = const_pool.tile([P, 1], f32, name="bm05")
            nc.gpsimd.memset(bm05[:], -0.5)

            # startup DMA order (one serialized DMA device): first x chunk,
            # then fp16 weight slabs in k-consumption order (slab 0 in
            # quarters, rest in halves so supply granularity keeps the PE
            # fed from first-matmul t~5.3us), chunk-1 x, then the fp8 pair
            # slabs consumed at the end of each chunk's K sweep
            xt_r = xt_d.rearrange("(ib p) n -> p ib n", p=P)
            w_r = w_d.rearrange("(f ib p) o -> p f ib o", p=P, f=NF16)
            w8_r = w8_d.rearrange("(pi two p) o -> p pi two o", p=P, two=2)
            x0 = xc_pool.tile([P, IB, NCHUNK], f16, tag="xr", name="x0")
            wt = wt_pool.tile([P, NF16, IB, O_SHARD], f16, name="wt")
            wt8 = wt_pool.tile([P, NPAIR, 2, O_SHARD], f8, name="wt8")
            nc.sync.dma_start(x0[:], xt_r[:, :, 0:NCHUNK])
            for q in range(4):
                nc.sync.dma_start(wt[:, 0, 2 * q:2 * q + 2], w_r[:, 0, 2 * q:2 * q + 2])
            H2 = IB // 2
            for f in range(1, NF16):
                nc.sync.dma_start(wt[:, f, :H2], w_r[:, f, :H2])
                nc.sync.dma_start(wt[:, f, H2:], w_r[:, f, H2:])
            wt8_g = lambda h: nc.sync.dma_start(wt8[:, 4 * h:4 * h + 4],
                                                w8_r[:, 4 * h:4 * h + 4])
            # prefetch the next three x chunks interleaved early so the
            # one-chunk-ahead clamp (below) never stalls the DVE queue
            xpre = {}
            def xfetch(c):
                t = xc_pool.tile([P, IB, NCHUNK], f16, tag="xr", name=f"x{c}")
                nc.sync.dma_start(t[:], xt_r[:, :, c * NCHUNK:(c + 1) * NCHUNK])
                xpre[c] = t
            wt8_g(0); wt8_g(1); wt8_g(2); wt8_g(3)
            xfetch(1)
            wt8_g(4); wt8_g(5)
            xfetch(2)
            wt8_g(6); wt8_g(7)
            xfetch(3)
            wt8_g(8)

            xcp = {}
            for chunk in range(CHUNKS):
                c0 = chunk * NCHUNK
                W = IB * NCHUNK  # 2048-wide feature ops
                if chunk == 0:
                    xr = x0
                elif chunk in xpre:
                    xr = xpre.pop(chunk)
                else:
                    xr = xc_pool.tile([P, IB, NCHUNK], f16, tag="xr", name="xr")
                    nc.sync.dma_start(xr[:], xt_r[:, :, c0:c0 + NCHUNK])
                xrf = xr[:].rearrange("p ib n -> p (ib n)")

                def plane(name):
                    return f_pool.tile([P, W], f16, tag=f"F_{name}", name=f"F_{name}")

                F = [None] * NF16  # [xc(phi1), r0(phi5), kL(phi4), kR(phi6)]
                if chunk in xcp:
                    F[0] = xcp.pop(chunk)
                else:
                    F[0] = plane("xc")
                    nc.vector.tensor_scalar(F[0][:], xrf, -1.0, 1.0, Op.max, Op.min)
                xc = F[0][:]
                # f16 chain first, ordered to match k-consumption:
                # ACT: x2, w2, v2 then silu/A2hi; DVE: x3, r0, kL, kR then
                # the fp8 Dekker prep planes
                x2 = plane("x2")
                nc.scalar.activation(x2[:], xc, Act.Square)
                x3 = plane("x3")
                nc.vector.tensor_tensor(x3[:], x2[:], xc, Op.mult)
                F[1] = plane("r0")  # phi5 = relu(xc^3)
                nc.vector.tensor_scalar(F[1][:], x3[:], 0.0, None, Op.max)
                w2 = tmp_pool.tile([P, W], f16, tag="sq", name="w2")
                nc.scalar.activation(w2[:], xc, Act.Square, bias=b05[:])
                wn = tmp_pool.tile([P, W], f16, tag="nn", name="wn")
                nc.vector.tensor_scalar(wn[:], xc, -1.0, 0.5, Op.mult, Op.subtract)
                F[2] = plane("kL")  # phi4
                nc.vector.scalar_tensor_tensor(F[2][:], wn[:], 0.0, w2[:],
                                               Op.max, Op.mult)
                v2 = tmp_pool.tile([P, W], f16, tag="sq", name="v2")
                nc.scalar.activation(v2[:], xc, Act.Square, bias=bm05[:])
                vn = tmp_pool.tile([P, W], f16, tag="nn", name="vn")
                nc.vector.tensor_scalar(vn[:], xc, -0.5, None, Op.add)
                F[3] = plane("kR")  # phi6
                nc.vector.scalar_tensor_tensor(F[3][:], vn[:], 0.0, v2[:],
                                               Op.max, Op.mult)
                # fp8 Dekker planes: A-pairs (hi, hi/4) for x2/x3, B lo4
                # planes, e sentinel pair, silu pair
                A2 = f_pool.tile([P, 2, W], f8, tag="A2", name="A2")
                nc.scalar.activation(A2[:, 0], xc, Act.Square)
                nc.vector.tensor_scalar(A2[:, 1], A2[:, 0], 0.25, None, Op.mult)
                A3 = f_pool.tile([P, 2, W], f8, tag="A3", name="A3")
                nc.gpsimd.tensor_copy(A3[:, 0], x3[:])
                nc.vector.tensor_scalar(A3[:, 1], A3[:, 0], 0.25, None, Op.mult)
                d2 = tmp_pool.tile([P, W], f16, tag="dd", name="d2")
                nc.vector.tensor_tensor(d2[:], x2[:], A2[:, 0], Op.subtract)
                L2 = f_pool.tile([P, W], f8, tag="L2", name="L2")
                nc.gpsimd.tensor_scalar(L2[:], d2[:], 4.0, None, Op.mult)
                d3 = tmp_pool.tile([P, W], f16, tag="dd", name="d3")
                nc.vector.tensor_tensor(d3[:], x3[:], A3[:, 0], Op.subtract)
                L3 = f_pool.tile([P, W], f8, tag="L3", name="L3")
                nc.gpsimd.tensor_scalar(L3[:], d3[:], 4.0, None, Op.mult)
                Fe = f_pool.tile([P, 2, W], f8, tag="Fe", name="Fe")
                nc.gpsimd.tensor_scalar(Fe[:, 0], xrf, 1.5, None, Op.is_ge)
                nc.vector.tensor_scalar(Fe[:, 1], xrf, 1.5, 0.0625, Op.is_ge, Op.mult)
                Fs = f_pool.tile([P, W], f8, tag="Fs", name="Fs")
                nc.scalar.activation(Fs[:], xc, Act.Silu)
                # next chunk's clamp at the END of this chunk's feature
                # block: ~17us ahead of its k=0 LDWEIGHTS without ever
                # stalling the in-order DVE queue on the x DMA
                if chunk + 1 < CHUNKS and chunk + 1 in xpre:
                    xn = xpre[chunk + 1][:].rearrange("p ib n -> p (ib n)")
                    t = plane("xc")
                    nc.vector.tensor_scalar(t[:], xn, -1.0, 1.0, Op.max, Op.min)
                    xcp[chunk + 1] = t

                # -- matmuls: 48 fp16 K-tiles then 16 fp8 DoubleRow pairs in
                # one PSUM accumulation group.  Chunk 0 runs k-major over
                # both 128-batch subtiles so each weight slab feeds two
                # matmuls as its DMA lands; later chunks run subtiles
                # serially so one group's eviction overlaps the next's
                # matmuls --
                L2v = L2[:].rearrange("p (ib n) -> p ib n", ib=IB)
                L3v = L3[:].rearrange("p (ib n) -> p ib n", ib=IB)
                Fsv = Fs[:].rearrange("p (ib n) -> p ib n", ib=IB)

                def dr_lhs(pi, ns):
                    # pair order: A2[ib]x8, A3[ib]x8, B2[j]x4, B3[j]x4,
                    # e[ib]x8, silu[ib]x8
                    if pi < 8:
                        o = pi * NCHUNK + ns * P
                        return A2[:, :, o:o + P]
                    if pi < 16:
                        o = (pi - 8) * NCHUNK + ns * P
                        return A3[:, :, o:o + P]
                    if pi < 20:
                        j = pi - 16
                        return L2v[:, 2 * j:2 * j + 2, ns * P:ns * P + P]
                    if pi < 24:
                        j = pi - 20
                        return L3v[:, 2 * j:2 * j + 2, ns * P:ns * P + P]
                    if pi < 32:
                        o = (pi - 24) * NCHUNK + ns * P
                        return Fe[:, :, o:o + P]
                    j = pi - 32
                    return Fsv[:, 2 * j:2 * j + 2, ns * P:ns * P + P]

                def sweep(ps, ns, o0, o1):
                    for k in range(KT16):
                        f, ib = divmod(k, IB)
                        off = ib * NCHUNK + ns * P
                        nc.tensor.matmul(
                            ps, F[f][:, off:off + P], wt[:, f, ib, o0:o1],
                            start=(k == 0), stop=False)
                    for pi in range(NPAIR):
                        nc.tensor.matmul(
                            ps, dr_lhs(pi, ns), wt8[:, pi, :, o0:o1],
                            start=False, stop=(pi == NPAIR - 1), perf_mode=DR)

                def evict(ps, ns):
                    o = out_pool.tile([P, O_SHARD], f32, tag="out", name="outt")
                    nc.vector.tensor_copy(o[:], ps[:])
                    r0 = c0 + ns * P
                    nc.sync.dma_start(y_d[r0:r0 + P, :], o[:])

                if chunk == 0:
                    pss = [psum_out.tile([P, O_SHARD], f32, tag=f"psout{ns}",
                                         name=f"psout{ns}", bufs=2)
                           for ns in range(NSUB)]
                    for k in range(KT16):
                        f, ib = divmod(k, IB)
                        for ns in range(NSUB):
                            off = ib * NCHUNK + ns * P
                            nc.tensor.matmul(
                                pss[ns][:], F[f][:, off:off + P], wt[:, f, ib],
                                start=(k == 0), stop=False)
                    for pi in range(NPAIR):
                        for ns in range(NSUB):
                            nc.tensor.matmul(
                                pss[ns][:], dr_lhs(pi, ns), wt8[:, pi],
                                start=False, stop=(pi == NPAIR - 1), perf_mode=DR)
                    for ns in range(NSUB):
                        evict(pss[ns], ns)
                elif chunk < CHUNKS - 1:
                    for ns in range(NSUB):
                        ps = psum_out.tile([P, O_SHARD], f32, tag=f"psout{ns}",
                                           name=f"psout{ns}", bufs=2)
                        sweep(ps[:], ns, 0, O_SHARD)
                        evict(ps, ns)
                else:
                    # last chunk: subtile 1 accumulates in shrinking column
                    # pieces so only the final small piece's eviction chain
                    # is exposed after the very last matmul
                    ps = psum_out.tile([P, O_SHARD], f32, tag="psout0",
                                       name="psout0", bufs=2)
                    sweep(ps[:], 0, 0, O_SHARD)
                    evict(ps, 0)
                    # exactly two pieces so each lands on its own PSUM
                    # buffer of the rotation: a start=True zeroes a whole
                    # bank, so pieces must never reuse a bank whose previous
                    # piece's eviction copy may still be in flight
                    PIECES = (384, 128)
                    off = 0
                    for h, HW in enumerate(PIECES):
                        psh = psum_out.tile([P, O_SHARD], f32, tag="psout1",
                                            name="psout1", bufs=2)[:, :HW]
                        sweep(psh, 1, off, off + HW)
                        o = out_pool.tile([P, HW], f32, tag=f"outl{h}",
                                          name=f"outl{h}")
                        nc.vector.tensor_copy(o[:], psh)
                        r0 = c0 + P
                        nc.sync.dma_start(y_d[r0:r0 + P, off:off + HW], o[:])
                        off += HW

    nc.compile()
    return nc


def _fold_weights(coeff, w_base):
    """Returns (Wt16 (KT16*P, D_OUT) f16, W8 (2*NPAIR*P, D_OUT) f8,
    host_bias (D_OUT,) f64)."""
    import ml_dtypes
    f8t = ml_dtypes.float8_e4m3

    def q8(a):
        return np.asarray(a).astype(f8t).astype(np.float64)

    T = _T48 / 48.0
    c64 = np.asarray(coeff).astype(np.float64)
    Wf = np.einsum('fb,oib->fio', T, c64)          # (7, i, o); Wf[0] = const

    # fp16 planes: phi1, phi4, phi5, phi6
    F16_PHIS = [1, 5, 4, 6]
    W16 = [Wf[f].astype(np.float16) for f in F16_PHIS]
    Wt16 = np.stack(W16).reshape(NF16, IB, P, D_OUT).reshape(KT16 * P, D_OUT)

    # 3-term Dekker for phi2 (x^2) and phi3 (x^3)
    def dek3(Wx):
        hi = np.asarray(Wx).astype(f8t)
        lo4 = ((Wx - q8(hi)) * 4.0).astype(f8t)
        b = (q8(hi) * 0.25).astype(f8t)
        return hi, lo4, b
    W2h, W2l, W2b = dek3(Wf[2])
    W3h, W3l, W3b = dek3(Wf[3])

    # edge feature cancels the 1^- limit against the DEVICE representation
    phi_at_1 = {1: 1.0, 4: 0.0, 5: 1.0, 6: 0.125}
    Wdev1 = sum(phi_at_1[f] * w.astype(np.float64)
                for f, w in zip(F16_PHIS, W16))
    # device x2/x3 at phi=1: hi=1, hi/4=0.25, lo4=0
    Wdev1 += q8(W2h) + 0.25 * q8(W2l)
    Wdev1 += q8(W3h) + 0.25 * q8(W3l)
    We = -(Wf[0] + Wdev1)
    Ws = np.asarray(w_base).astype(np.float64).T   # (i, o)

    def dekker16(Wx):
        hi = np.asarray(Wx).astype(f8t)
        lo = ((Wx - q8(hi)) * 16.0).astype(f8t)
        return hi, lo
    We_hi, We_lo = dekker16(We)
    Ws8 = np.asarray(Ws).astype(f8t)

    # pair-major packing, row k = (pi*2 + two)*P + p; pair order:
    # A2[ib]x8, A3[ib]x8, B2[j]x4, B3[j]x4, e[ib]x8, silu[ib]x8
    W8 = np.zeros((2 * NPAIR * P, D_OUT), dtype=f8t)
    def put(pi, a, b):
        W8[(2 * pi) * P:(2 * pi + 1) * P] = a
        W8[(2 * pi + 1) * P:(2 * pi + 2) * P] = b
    for ib in range(IB):
        r = slice(ib * P, (ib + 1) * P)
        put(ib, W2h[r], W2l[r])
        put(8 + ib, W3h[r], W3l[r])
        put(24 + ib, We_hi[r], We_lo[r])
    for j in range(4):
        put(16 + j, W2b[2 * j * P:(2 * j + 1) * P],
            W2b[(2 * j + 1) * P:(2 * j + 2) * P])
        put(20 + j, W3b[2 * j * P:(2 * j + 1) * P],
            W3b[(2 * j + 1) * P:(2 * j + 2) * P])
        put(32 + j, Ws8[2 * j * P:(2 * j + 1) * P],
            Ws8[(2 * j + 1) * P:(2 * j + 2) * P])

    host_bias = Wf[0].sum(axis=0)                  # (o,)
    return Wt16, W8, host_bias


def kernel(x, coeff, w_base, bias):
    global _PROGRAM
    from concourse.bass_utils import run_bass_kernel_spmd

    if _PROGRAM is None:
        _PROGRAM = _build_program()
    nc = _PROGRAM

    x = np.asarray(x, dtype=np.float32)
    # sentinel-encode the x>=1 edge cases as 2.0, then f16 (see _build_program)
    xs = np.where(x >= 1.0, np.float32(2.0),
                  np.clip(x, -1.0, 1.0)).astype(np.float16)
    Wt16, W8, host_bias = _fold_weights(coeff, w_base)
    badd = (host_bias + np.asarray(bias).astype(np.float64)).astype(np.float32)

    in_maps = []
    for core in range(8):
        cn, co = divmod(core, MESH_O)
        osl = slice(co * O_SHARD, (co + 1) * O_SHARD)
        in_maps.append({
            "xt": np.ascontiguousarray(xs[cn * N_SHARD:(cn + 1) * N_SHARD].T),
            "wt": np.ascontiguousarray(Wt16[:, osl]),
            "wt8": np.ascontiguousarray(W8[:, osl]),
        })

    res = run_bass_kernel_spmd(nc, in_maps, list(range(8)))

    y = np.empty((N_FULL, D_OUT), dtype=np.float32)
    for core in range(8):
        cn, co = divmod(core, MESH_O)
        y[cn * N_SHARD:(cn + 1) * N_SHARD, co * O_SHARD:(co + 1) * O_SHARD] = \
            res.results[core]["y"]
    y += badd[None, :]
    return y


# revision 42
# speedup vs baseline: 1.0003x; 1.0003x over previous
"""BSplineKAN layer kernel for 8 Trainium2 NeuronCores.

Math
----
Per element x (xc = clip(x, -1, 1)) the reference computes
    y[n,o] = sum_{i,b} basis_b(xc[n,i]) * coeff[o,i,b] + silu(xc) @ w_base.T + bias
with the 7-function clamped cubic B-spline basis on knots
{-1(x4), -0.5, 0, 0.5, 1(x4)}.  Reference quirk: at xc == 1.0 exactly the
basis row is all ZERO.

On [-1, 1) the basis lives in the 7-dim space spanned by the truncated-power
features of xc
    phi = [1, x, x^2, x^3, relu(-x-0.5)^3, relu(x)^3, relu(x-0.5)^3]
(exact integer/48 conversion matrix T, hardcoded below).  Linear tricks
remove all masking from the device:
  * the constant feature's contribution sum_i W0[i,o] is added on the HOST
    (with the input bias), so phi0 never hits the matmul;
  * the xc == 1 edge case becomes one extra linear feature e = (x >= 1)
    whose weights cancel the spline's 1^- limit:
    W_e = -(W0 + sum_f phi_f(1) * fp16(W_f)), folded against the fp16-rounded
    weights so the cancellation is exact at working resolution.  The edge
    decision is made on the HOST in f32 (f16 rounding must not flip the
    branch) and shipped as a 2.0 sentinel inside the f16 x tensor.

Contraction layout (K = 8 * 1024):
  * 6 truncated-power features run as 48 fp16 K-tiles (their weights carry
    ~5x term cancellation, so fp8 anywhere on them measures 3-25e-2 error
    vs the 2e-2 gate -- evaluated and rejected);
  * e and silu run as 16 fp8e4 DoubleRow pairs (0.5 cycles/row) carrying
    Dekker-split weights: pair = (F, F/16) x (q8(W), q8(16*(W - q8(W)))),
    giving ~fp16 weight precision at half the PE cost.  e is 0/1 (fp8
    exact); silu's fp8 feature rounding adds ~2e-3 relative error.
  PE time: 48*512 + 16*256 cycles per 128-batch group = 191us/core vs
  300us for the 11-feature fp16 baseline.

Distribution: 4-way batch x 2-way d_out mesh over 8 cores.  Per core:
x host-encoded (1024, 2048) f16, weights (6144, 512) f16 + (4096, 512) f8
resident in SBUF, output (2048, 512) f32; y += host bias after gather.
"""

import numpy as np

# ---- problem constants (hardcoded per contract) ----
N_FULL, D_IN, D_OUT = 8192, 1024, 1024
MESH_N, MESH_O = 4, 2                 # 4-way batch x 2-way d_out
N_SHARD = N_FULL // MESH_N            # 2048
O_SHARD = D_OUT // MESH_O             # 512
P = 128
NF16 = 4                              # fp16 feature planes (phi_1,4,5,6)
IB = D_IN // P                        # 8 i-blocks
KT16 = NF16 * IB                      # 32 fp16 K-tiles
# fp8 DoubleRow pairs: A-pairs x2, x3 (8+8), B-pairs x2, x3 (4+4),
# e (8, Dekker weights), silu (4, cross-ib pairs w/ single fp8 weights --
# w_base sigma 1/32 tolerates plain fp8) = 36
NPAIR = 36
NCHUNK = 256                          # batch cols per pipeline chunk
WARMUP = 120                          # PE clock-ramp filler matmuls
NSUB = NCHUNK // P                    # 2
CHUNKS = N_SHARD // NCHUNK            # 8

# basis_b(x) = sum_f T48[f, b]/48 * phi_f(x) on [-1, 1),
# phi = [1, x, x^2, x^3, relu(-x-0.5)^3, relu(x)^3, relu(x-0.5)^3]
_T48 = np.array([
    [0,    0,    8,    32,   8,    0,    0],
    [0,    0,   -48,   0,    48,   0,    0],
    [0,    0,    96,  -192,  96,   0,    0],
    [0,   -96,   224, -192,  64,   0,    0],
    [384, -768,  576, -256,  64,   0,    0],
    [0,    96,  -288,  384, -288,  96,   0],
    [0,    0,    64,  -256,  576, -768,  384],
], dtype=np.float64)

# fp16 plane order (K-tile index f*IB + ib); phi index it maps to:
#   f0: xc (phi1)   f1: xc^2 (phi2)        f2: xc^3 (phi3)
#   f3: relu(-(x+.5)^3) (phi4)  f4: relu(x^3) (phi5)  f5: relu((x-.5)^3) (phi6)
_PHI_AT_1 = np.array([1.0, 1.0, 1.0, 0.0, 1.0, 0.125])  # phi_{1..6}(1)

_PROGRAM = None  # compiled Bass program, built once


def _build_program():
    import concourse.mybir as mybir
    import concourse.tile as tile
    from concourse import bacc

    f32 = mybir.dt.float32
    f16 = mybir.dt.float16
    f8 = mybir.dt.float8e4
    Op = mybir.AluOpType
    Act = mybir.ActivationFunctionType
    DR = mybir.MatmulPerfMode.DoubleRow

    nc = bacc.Bacc("TRN2", target_bir_lowering=False, debug=False)
    xt_d = nc.dram_tensor("xt", [D_IN, N_SHARD], f16, kind="ExternalInput").ap()
    w_d = nc.dram_tensor("wt", [KT16 * P, O_SHARD], f16, kind="ExternalInput").ap()
    w8_d = nc.dram_tensor("wt8", [2 * NPAIR * P, O_SHARD], f8,
                          kind="ExternalInput").ap()
    y_d = nc.dram_tensor("y", [N_SHARD, O_SHARD], f32, kind="ExternalOutput").ap()

    with tile.TileContext(nc) as tc:
        with (
            tc.tile_pool(name="const", bufs=1) as const_pool,
            tc.tile_pool(name="wt", bufs=1) as wt_pool,
            tc.tile_pool(name="feat", bufs=2) as f_pool,
            tc.tile_pool(name="xc", bufs=3) as xc_pool,
            tc.tile_pool(name="tmp", bufs=2) as tmp_pool,
            tc.tile_pool(name="out", bufs=2) as out_pool,
            tc.tile_pool(name="pso", bufs=4, space="PSUM") as psum_out,
        ):
            # warm-up tile memset FIRST so PE warm-up matmuls start asap and
            # anchor the p-state clock ramp; they fill until the first weight
            # slab + x chunk land (~5.3us)
            wz = const_pool.tile([P, P], f16, name="wz")
            nc.gpsimd.memset(wz[:], 0.0)
            pw = psum_out.tile([P, 64], f32, tag="pwarm", name="pwarm", bufs=1)
            for i in range(WARMUP):
                nc.tensor.matmul(pw[:], wz[:], wz[:, :64],
                                 start=(i == 0), stop=(i == WARMUP - 1))

            # tiny dummy activations so both ACT table sets load during the
            # initial DMA wait instead of on the first feature's critical path
            warm = const_pool.tile([P, 1], f32, name="warm")
            nc.gpsimd.memset(warm[:], 0.0)
            nc.scalar.activation(warm[:], warm[:], Act.Square)
            nc.scalar.activation(warm[:], warm[:], Act.Silu)
            b05 = const_pool.tile([P, 1], f32, name="b05")
            nc.gpsimd.memset(b05[:], 0.5)
            bm05 = const_pool.tile([P, 1], f32, name="bm05")
            nc.gpsimd.memset(bm05[:], -0.5)

            # startup DMA order (one serialized DMA device): first x chunk,
            # then fp16 weight slabs in k-consumption order (slab 0 in
            # quarters, rest in halves so supply granularity keeps the PE
            # fed from first-matmul t~5.3us), chunk-1 x, then the fp8 pair
            # slabs consumed at the end of each chunk's K sweep
            xt_r = xt_d.rearrange("(ib p) n -> p ib n", p=P)
            w_r = w_d.rearrange("(f ib p) o -> p f ib o", p=P, f=NF16)
            w8_r = w8_d.rearrange("(pi two p) o -> p pi two o", p=P, two=2)
            x0 = xc_pool.tile([P, IB, NCHUNK], f16, tag="xr", name="x0")
            wt = wt_pool.tile([P, NF16, IB, O_SHARD], f16, name="wt")
            wt8 = wt_pool.tile([P, NPAIR, 2, O_SHARD], f8, name="wt8")
            nc.sync.dma_start(x0[:], xt_r[:, :, 0:NCHUNK])
            for q in range(4):
                nc.sync.dma_start(wt[:, 0, 2 * q:2 * q + 2], w_r[:, 0, 2 * q:2 * q + 2])
            H2 = IB // 2
            for f in range(1, NF16):
                nc.sync.dma_start(wt[:, f, :H2], w_r[:, f, :H2])
                nc.sync.dma_start(wt[:, f, H2:], w_r[:, f, H2:])
            wt8_g = lambda h: nc.sync.dma_start(wt8[:, 4 * h:4 * h + 4],
                                                w8_r[:, 4 * h:4 * h + 4])
            # prefetch the next three x chunks interleaved early so the
            # one-chunk-ahead clamp (below) never stalls the DVE queue
            xpre = {}
            def xfetch(c):
                t = xc_pool.tile([P, IB, NCHUNK], f16, tag="xr", name=f"x{c}")
                nc.sync.dma_start(t[:], xt_r[:, :, c * NCHUNK:(c + 1) * NCHUNK])
                xpre[c] = t
            wt8_g(0); wt8_g(1); wt8_g(2); wt8_g(3)
            xfetch(1)
            wt8_g(4); wt8_g(5)
            xfetch(2)
            wt8_g(6); wt8_g(7)
            xfetch(3)
            wt8_g(8)

            xcp = {}
            for chunk in range(CHUNKS):
                c0 = chunk * NCHUNK
                W = IB * NCHUNK  # 2048-wide feature ops
                if chunk == 0:
                    xr = x0
                elif chunk in xpre:
                    xr = xpre.pop(chunk)
                else:
                    xr = xc_pool.tile([P, IB, NCHUNK], f16, tag="xr", name="xr")
                    nc.sync.dma_start(xr[:], xt_r[:, :, c0:c0 + NCHUNK])
                xrf = xr[:].rearrange("p ib n -> p (ib n)")

                def plane(name):
                    return f_pool.tile([P, W], f16, tag=f"F_{name}", name=f"F_{name}")

                F = [None] * NF16  # [xc(phi1), r0(phi5), kL(phi4), kR(phi6)]
                if chunk in xcp:
                    F[0] = xcp.pop(chunk)
                else:
                    F[0] = plane("xc")
                    nc.vector.tensor_scalar(F[0][:], xrf, -1.0, 1.0, Op.max, Op.min)
                xc = F[0][:]
                # f16 chain first, ordered to match k-consumption:
                # ACT: x2, w2, v2 then silu/A2hi; DVE: x3, r0, kL, kR then
                # the fp8 Dekker prep planes
                x2 = plane("x2")
                nc.scalar.activation(x2[:], xc, Act.Square)
                x3 = plane("x3")
                nc.vector.tensor_tensor(x3[:], x2[:], xc, Op.mult)
                F[1] = plane("r0")  # phi5 = relu(xc^3)
                nc.vector.tensor_scalar(F[1][:], x3[:], 0.0, None, Op.max)
                w2 = tmp_pool.tile([P, W], f16, tag="sq", name="w2")
                nc.scalar.activation(w2[:], xc, Act.Square, bias=b05[:])
                wn = tmp_pool.tile([P, W], f16, tag="nn", name="wn")
                nc.vector.tensor_scalar(wn[:], xc, -1.0, 0.5, Op.mult, Op.subtract)
                F[2] = plane("kL")  # phi4
                nc.vector.scalar_tensor_tensor(F[2][:], wn[:], 0.0, w2[:],
                                               Op.max, Op.mult)
                v2 = tmp_pool.tile([P, W], f16, tag="sq", name="v2")
                nc.scalar.activation(v2[:], xc, Act.Square, bias=bm05[:])
                vn = tmp_pool.tile([P, W], f16, tag="nn", name="vn")
                nc.vector.tensor_scalar(vn[:], xc, -0.5, None, Op.add)
                F[3] = plane("kR")  # phi6
                nc.vector.scalar_tensor_tensor(F[3][:], vn[:], 0.0, v2[:],
                                               Op.max, Op.mult)
                # fp8 Dekker planes: A-pairs (hi, hi/4) for x2/x3, B lo4
                # planes, e sentinel pair, silu pair
                A2 = f_pool.tile([P, 2, W], f8, tag="A2", name="A2")
                nc.scalar.activation(A2[:, 0], xc, Act.Square)
                nc.vector.tensor_scalar(A2[:, 1], A2[:, 0], 0.25, None, Op.mult)
                A3 = f_pool.tile([P, 2, W], f8, tag="A3", name="A3")
                nc.gpsimd.tensor_copy(A3[:, 0], x3[:])
                nc.vector.tensor_scalar(A3[:, 1], A3[:, 0], 0.25, None, Op.mult)
                d2 = tmp_pool.tile([P, W], f16, tag="dd", name="d2")
                nc.vector.tensor_tensor(d2[:], x2[:], A2[:, 0], Op.subtract)
                L2 = f_pool.tile([P, W], f8, tag="L2", name="L2")
                nc.gpsimd.tensor_scalar(L2[:], d2[:], 4.0, None, Op.mult)
                d3 = tmp_pool.tile([P, W], f16, tag="dd", name="d3")
                nc.vector.tensor_tensor(d3[:], x3[:], A3[:, 0], Op.subtract)
                L3 = f_pool.tile([P, W], f8, tag="L3", name="L3")
                nc.gpsimd.tensor_scalar(L3[:], d3[:], 4.0, None, Op.mult)
                Fe = f_pool.tile([P, 2, W], f8, tag="Fe", name="Fe")
                nc.gpsimd.tensor_scalar(Fe[:, 0], xrf, 1.5, None, Op.is_ge)
                nc.vector.tensor_scalar(Fe[:, 1], xrf, 1.5, 0.0625, Op.is_ge, Op.mult)
                Fs = f_pool.tile([P, W], f8, tag="Fs", name="Fs")
                nc.scalar.activation(Fs[:], xc, Act.Silu)
                # next chunk's clamp at the END of this chunk's feature
                # block: ~17us ahead of its k=0 LDWEIGHTS without ever
                # stalling the in-order DVE queue on the x DMA
                if chunk + 1 < CHUNKS and chunk + 1 in xpre:
                    xn = xpre[chunk + 1][:].rearrange("p ib n -> p (ib n)")
                    t = plane("xc")
                    nc.vector.tensor_scalar(t[:], xn, -1.0, 1.0, Op.max, Op.min)
                    xcp[chunk + 1] = t

                # -- matmuls: 48 fp16 K-tiles then 16 fp8 DoubleRow pairs in
                # one PSUM accumulation group.  Chunk 0 runs k-major over
                # both 128-batch subtiles so each weight slab feeds two
                # matmuls as its DMA lands; later chunks run subtiles
                # serially so one group's eviction overlaps the next's
                # matmuls --
                L2v = L2[:].rearrange("p (ib n) -> p ib n", ib=IB)
                L3v = L3[:].rearrange("p (ib n) -> p ib n", ib=IB)
                Fsv = Fs[:].rearrange("p (ib n) -> p ib n", ib=IB)

                def dr_lhs(pi, ns):
                    # pair order: A2[ib]x8, A3[ib]x8, B2[j]x4, B3[j]x4,
                    # e[ib]x8, silu[ib]x8
                    if pi < 8:
                        o = pi * NCHUNK + ns * P
                        return A2[:, :, o:o + P]
                    if pi < 16:
                        o = (pi - 8) * NCHUNK + ns * P
                        return A3[:, :, o:o + P]
                    if pi < 20:
                        j = pi - 16
                        return L2v[:, 2 * j:2 * j + 2, ns * P:ns * P + P]
                    if pi < 24:
                        j = pi - 20
                        return L3v[:, 2 * j:2 * j + 2, ns * P:ns * P + P]
                    if pi < 32:
                        o = (pi - 24) * NCHUNK + ns * P
                        return Fe[:, :, o:o + P]
                    j = pi - 32
                    return Fsv[:, 2 * j:2 * j + 2, ns * P:ns * P + P]

                def sweep(ps, ns, o0, o1):
                    for k in range(KT16):
                        f, ib = divmod(k, IB)
                        off = ib * NCHUNK + ns * P
                        nc.tensor.matmul(
                            ps, F[f][:, off:off + P], wt[:, f, ib, o0:o1],
                            start=(k == 0), stop=False)
                    for pi in range(NPAIR):
                        nc.tensor.matmul(
                            ps, dr_lhs(pi, ns), wt8[:, pi, :, o0:o1],
                            start=False, stop=(pi == NPAIR - 1), perf_mode=DR)

                def evict(ps, ns):
                    o = out_pool.tile([P, O_SHARD], f32, tag="out", name="outt")
                    nc.vector.tensor_copy(o[:], ps[:])
                    r0 = c0 + ns * P
                    nc.sync.dma_start(y_d[r0:r0 + P, :], o[:])

                if chunk == 0:
                    pss = [psum_out.tile([P, O_SHARD], f32, tag=f"psout{ns}",
                                         name=f"psout{ns}", bufs=2)
                           for ns in range(NSUB)]
                    for k in range(KT16):
                        f, ib = divmod(k, IB)
                        for ns in range(NSUB):
                            off = ib * NCHUNK + ns * P
                            nc.tensor.matmul(
                                pss[ns][:], F[f][:, off:off + P], wt[:, f, ib],
                                start=(k == 0), stop=False)
                    for pi in range(NPAIR):
                        for ns in range(NSUB):
                            nc.tensor.matmul(
                                pss[ns][:], dr_lhs(pi, ns), wt8[:, pi],
                                start=False, stop=(pi == NPAIR - 1), perf_mode=DR)
                    for ns in range(NSUB):
                        evict(pss[ns], ns)
                elif chunk < CHUNKS - 1:
                    for ns in range(NSUB):
                        ps = psum_out.tile([P, O_SHARD], f32, tag=f"psout{ns}",
                                           name=f"psout{ns}", bufs=2)
                        sweep(ps[:], ns, 0, O_SHARD)
                        evict(ps, ns)
                else:
                    # last chunk: subtile 1 accumulates in shrinking column
                    # pieces so only the final small piece's eviction chain
                    # is exposed after the very last matmul
                    ps = psum_out.tile([P, O_SHARD], f32, tag="psout0",
                                       name="psout0", bufs=2)
                    sweep(ps[:], 0, 0, O_SHARD)
                    evict(ps, 0)
                    # exactly two pieces so each lands on its own PSUM
                    # buffer of the rotation: a start=True zeroes a whole
                    # bank, so pieces must never reuse a bank whose previous
                    # piece's eviction copy may still be in flight
                    PIECES = (448, 64)
                    off = 0
                    for h, HW in enumerate(PIECES):
                        psh = psum_out.tile([P, O_SHARD], f32, tag="psout1",
                                            name="psout1", bufs=2)[:, :HW]
                        sweep(psh, 1, off, off + HW)
                        o = out_pool.tile([P, HW], f32, tag=f"outl{h}",
                                          name=f"outl{h}")
                        nc.vector.tensor_copy(o[:], psh)
                        r0 = c0 + P
                        nc.sync.dma_start(y_d[r0:r0 + P, off:off + HW], o[:])
                        off += HW

    nc.compile()
    return nc


def _fold_weights(coeff, w_base):
    """Returns (Wt16 (KT16*P, D_OUT) f16, W8 (2*NPAIR*P, D_OUT) f8,
    host_bias (D_OUT,) f64)."""
    import ml_dtypes
    f8t = ml_dtypes.float8_e4m3

    def q8(a):
        return np.asarray(a).astype(f8t).astype(np.float64)

    T = _T48 / 48.0
    c64 = np.asarray(coeff).astype(np.float64)
    Wf = np.einsum('fb,oib->fio', T, c64)          # (7, i, o); Wf[0] = const

    # fp16 planes: phi1, phi4, phi5, phi6
    F16_PHIS = [1, 5, 4, 6]
    W16 = [Wf[f].astype(np.float16) for f in F16_PHIS]
    Wt16 = np.stack(W16).reshape(NF16, IB, P, D_OUT).reshape(KT16 * P, D_OUT)

    # 3-term Dekker for phi2 (x^2) and phi3 (x^3)
    def dek3(Wx):
        hi = np.asarray(Wx).astype(f8t)
        lo4 = ((Wx - q8(hi)) * 4.0).astype(f8t)
        b = (q8(hi) * 0.25).astype(f8t)
        return hi, lo4, b
    W2h, W2l, W2b = dek3(Wf[2])
    W3h, W3l, W3b = dek3(Wf[3])

    # edge feature cancels the 1^- limit against the DEVICE representation
    phi_at_1 = {1: 1.0, 4: 0.0, 5: 1.0, 6: 0.125}
    Wdev1 = sum(phi_at_1[f] * w.astype(np.float64)
                for f, w in zip(F16_PHIS, W16))
    # device x2/x3 at phi=1: hi=1, hi/4=0.25, lo4=0
    Wdev1 += q8(W2h) + 0.25 * q8(W2l)
    Wdev1 += q8(W3h) + 0.25 * q8(W3l)
    We = -(Wf[0] + Wdev1)
    Ws = np.asarray(w_base).astype(np.float64).T   # (i, o)

    def dekker16(Wx):
        hi = np.asarray(Wx).astype(f8t)
        lo = ((Wx - q8(hi)) * 16.0).astype(f8t)
        return hi, lo
    We_hi, We_lo = dekker16(We)
    Ws8 = np.asarray(Ws).astype(f8t)

    # pair-major packing, row k = (pi*2 + two)*P + p; pair order:
    # A2[ib]x8, A3[ib]x8, B2[j]x4, B3[j]x4, e[ib]x8, silu[ib]x8
    W8 = np.zeros((2 * NPAIR * P, D_OUT), dtype=f8t)
    def put(pi, a, b):
        W8[(2 * pi) * P:(2 * pi + 1) * P] = a
        W8[(2 * pi + 1) * P:(2 * pi + 2) * P] = b
    for ib in range(IB):
        r = slice(ib * P, (ib + 1) * P)
        put(ib, W2h[r], W2l[r])
        put(8 + ib, W3h[r], W3l[r])
        put(24 + ib, We_hi[r], We_lo[r])
    for j in range(4):
        put(16 + j, W2b[2 * j * P:(2 * j + 1) * P],
            W2b[(2 * j + 1) * P:(2 * j + 2) * P])
        put(20 + j, W3b[2 * j * P:(2 * j + 1) * P],
            W3b[(2 * j + 1) * P:(2 * j + 2) * P])
        put(32 + j, Ws8[2 * j * P:(2 * j + 1) * P],
            Ws8[(2 * j + 1) * P:(2 * j + 2) * P])

    host_bias = Wf[0].sum(axis=0)                  # (o,)
    return Wt16, W8, host_bias


def kernel(x, coeff, w_base, bias):
    global _PROGRAM
    from concourse.bass_utils import run_bass_kernel_spmd

    if _PROGRAM is None:
        _PROGRAM = _build_program()
    nc = _PROGRAM

    x = np.asarray(x, dtype=np.float32)
    # sentinel-encode the x>=1 edge cases as 2.0, then f16 (see _build_program)
    xs = np.where(x >= 1.0, np.float32(2.0),
                  np.clip(x, -1.0, 1.0)).astype(np.float16)
    Wt16, W8, host_bias = _fold_weights(coeff, w_base)
    badd = (host_bias + np.asarray(bias).astype(np.float64)).astype(np.float32)

    in_maps = []
    for core in range(8):
        cn, co = divmod(core, MESH_O)
        osl = slice(co * O_SHARD, (co + 1) * O_SHARD)
        in_maps.append({
            "xt": np.ascontiguousarray(xs[cn * N_SHARD:(cn + 1) * N_SHARD].T),
            "wt": np.ascontiguousarray(Wt16[:, osl]),
            "wt8": np.ascontiguousarray(W8[:, osl]),
        })

    res = run_bass_kernel_spmd(nc, in_maps, list(range(8)))

    y = np.empty((N_FULL, D_OUT), dtype=np.float32)
    for core in range(8):
        cn, co = divmod(core, MESH_O)
        y[cn * N_SHARD:(cn + 1) * N_SHARD, co * O_SHARD:(co + 1) * O_SHARD] = \
            res.results[core]["y"]
    y += badd[None, :]
    return y
